# revision 19
# baseline (speedup 1.0000x reference)
"""Multi-head attention (b=2, n=2048, d=1024, h=16) on 8 TRN2 NeuronCores.

Sharding: tensor-parallel 8-way over heads — core c handles heads 2c..2c+1
(channel rows 128c..128c+127) for BOTH batches. Column-parallel QKV.

Token ownership is strip-interleaved: core g owns tokens {512k + 128g + r}
of its batch. Attention i-block (b,k) covers the CONTIGUOUS batch-b tokens
[512k, 512k+512), which holds a 128-token strip for each of 4 owner cores,
so after blocks (0,k) and (1,k) one 8-core 256KB AllToAll exchanges exactly
the [128ch, 128tok] tiles every peer needs (pipelined under compute; ~4x
less wire than a ReduceScatter of row-parallel partials). Each core then
runs the full 1024-channel output projection for its own 512 tokens locally
with f32 PSUM accumulation and writes final f32 output directly.

The whole kernel is one software-pipelined stream over 128 (block, j)
iterations: per slot the tensor engine gets scores (2 matmuls), the lagged
AV pair, and one deadline-scheduled "filler" unit (batch-1 QKV projection
chains, V tiles, output projections). Dense back-to-back PE work keeps the
PE at its full 2.4GHz P-state (bursty execution throttles it to 1.2GHz).
Softmax normalization runs entirely off the PE: denominators leave PSUM via
one DVE copy, reciprocal on DVE, partition-broadcast on the idle GPSIMD,
multiply on DVE. Exp (scalar engine, ~137us) and PE (~165us) are both kept
near-saturated.

Matmul operands are bf16 (fp32 PSUM accumulation); softmax statistics and
normalization run in fp32. Host-side prep is layout-only (slicing/
transpose/dtype).
"""

import sys
from contextlib import ExitStack

_TRN_REPO = "/opt/trn_rl_repo"
if _TRN_REPO not in sys.path:
    sys.path.insert(0, _TRN_REPO)

import ml_dtypes
import numpy as np

import concourse.bass as bass
import concourse.bacc as bacc
import concourse.tile as tile
from concourse import mybir

F32 = mybir.dt.float32
BF16 = mybir.dt.bfloat16

B = 2          # batch
N = 2048       # tokens per batch
U = B * N      # total token columns per core (both batches)
D = 1024       # model dim
H = 16         # heads
HD = D // H    # 64 head dim
N_CORES = 8
GROUP = [[0, 1, 2, 3, 4, 5, 6, 7]]
HPC = 2        # heads per core
CPC = HPC * HD  # 128 channels per core
BW = 512       # attention i-block width (tokens)
NBLK = N // BW  # 4 strips per batch
NJ = N // 128   # 16 key tiles per batch
AV_LAG = 3      # AV trails exp by this many pipeline slots


def build_program():
    nc = bacc.Bacc("TRN2", target_bir_lowering=False, debug=False,
                   num_devices=N_CORES)

    # ---- DRAM I/O (per-core shards, host-prepared, bf16) ----
    xt_d = nc.dram_tensor("xt", [D, U], BF16, kind="ExternalInput").ap()
    wqt_d = nc.dram_tensor("wqt", [D, CPC], BF16, kind="ExternalInput").ap()
    wkt_d = nc.dram_tensor("wkt", [D, CPC], BF16, kind="ExternalInput").ap()
    wvt_d = nc.dram_tensor("wvt", [D, CPC], BF16, kind="ExternalInput").ap()
    wot_d = nc.dram_tensor("wot", [D, D], BF16, kind="ExternalInput").ap()
    bo_d = nc.dram_tensor("bob", [128, D], F32, kind="ExternalInput").ap()
    out_d = nc.dram_tensor("out", [512, D], F32, kind="ExternalOutput").ap()

    osend_d = [nc.dram_tensor(f"osend{k}", [8 * CPC, 128], BF16).ap()
               for k in range(NBLK)]
    orecv_d = [nc.dram_tensor(f"orecv{k}", [8 * CPC, 128], BF16).ap()
               for k in range(NBLK)]
    warm_i = nc.dram_tensor("warm_i", [8, 4], BF16).ap()
    warm_o = nc.dram_tensor("warm_o", [8, 4], BF16).ap()

    with tile.TileContext(nc) as tc, ExitStack() as octx:
        wpool = octx.enter_context(tc.tile_pool(name="wpool", bufs=1))
        qk_pool = octx.enter_context(tc.tile_pool(name="qk", bufs=1))
        v_pool = octx.enter_context(tc.tile_pool(name="vaug", bufs=1))
        o_pool = octx.enter_context(tc.tile_pool(name="opair", bufs=1))
        xt_pool = octx.enter_context(tc.tile_pool(name="xt", bufs=1))
        st_pool = octx.enter_context(tc.tile_pool(name="stp", bufs=8))
        nrm_pool = octx.enter_context(tc.tile_pool(name="nrm", bufs=4))
        og_pool = octx.enter_context(tc.tile_pool(name="og", bufs=2))
        fin_pool = octx.enter_context(tc.tile_pool(name="fin", bufs=4))
        # PSUM banks: st 2x[128,1024]f32 = 4, ot 2x[65,512] = 2,
        #             mm 1x[128,512] = 1, bps 1x[64,512] = 1
        mm_ps = octx.enter_context(tc.tile_pool(name="mmps", bufs=1, space="PSUM"))
        st_ps_pool = octx.enter_context(
            tc.tile_pool(name="stps", bufs=2, space="PSUM"))
        ot_ps = octx.enter_context(tc.tile_pool(name="otps", bufs=2, space="PSUM"))
        bps_ps = octx.enter_context(tc.tile_pool(name="bpsp", bufs=1, space="PSUM"))

        # ---- prologue: DMAs + collective warmup ----
        warm_sb = wpool.tile([8, 4], BF16, tag="warm")
        nc.gpsimd.memset(warm_sb[:], 1.0)
        nc.sync.dma_start(warm_i[:], warm_sb[:])
        nc.gpsimd.collective_compute(
            "AllToAll", mybir.AluOpType.bypass, replica_groups=GROUP,
            ins=[warm_i[:]], outs=[warm_o[:]])

        def load_w(name, dram, rows, cols):
            nch = rows // 128
            raw = wpool.tile([128, nch * cols], BF16, tag=name, name=name + "_t")
            nc.sync.dma_start(
                raw[:].rearrange("p (c m) -> p c m", c=nch),
                dram.rearrange("(c p) m -> p c m", p=128))
            return raw[:]

        wqt = load_w("wqt", wqt_d, D, CPC)
        wkt = load_w("wkt", wkt_d, D, CPC)
        wvt = load_w("wvt", wvt_d, D, CPC)

        # x^T in [128, 1024]-column chunks, batch-0 halves first, so the
        # first score matmuls start after ~2.7MB of input traffic
        xt_sb = [xt_pool.tile([128, U], BF16, tag=f"xtr{ch}", name=f"xtr{ch}")
                 for ch in range(8)]
        for half in range(4):
            for ch in range(8):
                nc.sync.dma_start(
                    xt_sb[ch][:, 1024 * half: 1024 * (half + 1)],
                    xt_d[128 * ch:128 * (ch + 1), 1024 * half: 1024 * (half + 1)])

        # big weights the PE only needs late go on the gpsimd DMA queue
        nchw = D // 128
        wot_raw = wpool.tile([128, nchw * D], BF16, tag="wot", name="wot_t")
        nc.gpsimd.dma_start(
            wot_raw[:].rearrange("p (c m) -> p c m", c=nchw),
            wot_d.rearrange("(c p) m -> p c m", p=128))
        wot = wot_raw[:]
        bias_sb = wpool.tile([128, D], F32, tag="bias")
        nc.gpsimd.dma_start(bias_sb[:], bo_d[:])

        # q/k in [128ch, U] layout: rows = 2 heads x 64 hd, col u = 2048b + t
        qtp = qk_pool.tile([128, U], BF16, tag="qtp", name="qtp")
        ktp = qk_pool.tile([128, U], BF16, tag="ktp", name="ktp")
        # V per (batch, key-tile): [128 keys, 2 heads x (64 hd | ones col)].
        # Ones columns come from an f32 memset + DVE copy (a direct bf16
        # memset writes a garbled constant on this stack).
        ones_f = wpool.tile([128, 64], F32, tag="ones_f")
        nc.gpsimd.memset(ones_f[:], 1.0)
        ones1 = wpool.tile([1, 64], BF16, tag="ones1")
        nc.vector.tensor_copy(ones1[:], ones_f[0:1, :])
        vaug = [[v_pool.tile([128, HPC * 65], BF16, tag=f"va{b}_{j}",
                             name=f"va{b}_{j}")
                 for j in range(NJ)] for b in range(B)]
        for b in range(B):
            for j in range(NJ):
                nc.vector.tensor_copy(
                    vaug[b][j][:].rearrange("p (h m) -> p h m",
                                            h=HPC)[:, :, 64:65],
                    ones_f[:].rearrange("p (h m) -> p h m", m=1)[:, 0:HPC, :])
        # O in [128ch, U] layout, same column indexing as q/k
        opair = o_pool.tile([128, U], BF16, tag="opr", name="opr")

        # PE keep-alive: the PE drops from 2.4GHz to 1.2GHz after ANY idle
        # and needs ~3us of continuous work to ramp back. Tiny self-contained
        # f32 matmuls on ones_f (no data deps, no readers) bridge the idle
        # moments so the pipeline's real matmuls run at full clock.
        def dummy_mm(n=1):
            dt_ = bps_ps.tile([64, BW], F32, tag="bps")
            for _ in range(n):
                nc.tensor.matmul(dt_[0:1, 0:64], ones_f[:, 0:1],
                                 ones_f[:, 0:64], start=True, stop=True)

        # ramp the clock while the input DMAs stream in
        dummy_mm(56)

        scale = float(HD) ** -0.5

        # ---- PE work generators (each yield = one pipeline slot) ----
        def gen_qk_ic(wmat, dst, b, ic):
            """One (q|k, batch, 512-col block) projection chain: 8 matmuls
            accumulating over d, split across 2 slots, then a DVE cast."""
            c0 = 2048 * b + 512 * ic
            ps = mm_ps.tile([128, 512], F32, tag="mm")
            for ch in range(4):
                nc.tensor.matmul(
                    ps[:],
                    wmat[:, 128 * ch: 128 * (ch + 1)],
                    xt_sb[ch][:, c0: c0 + 512],
                    start=(ch == 0), stop=False)
            yield
            for ch in range(4, 8):
                nc.tensor.matmul(
                    ps[:],
                    wmat[:, 128 * ch: 128 * (ch + 1)],
                    xt_sb[ch][:, c0: c0 + 512],
                    start=False, stop=(ch == 7))
            nc.vector.tensor_copy(dst[:, c0: c0 + 512], ps[:])
            yield

        def gen_v(b, j):
            """V for batch b, key tile j: 8 F=128 matmuls + cast, one slot."""
            ps = mm_ps.tile([128, 512], F32, tag="mm")
            for ch in range(8):
                nc.tensor.matmul(
                    ps[:, 0:128],
                    xt_sb[ch][:, 2048 * b + 128 * j: 2048 * b + 128 * (j + 1)],
                    wvt[:, 128 * ch: 128 * (ch + 1)],
                    start=(ch == 0), stop=(ch == 7))
            nc.vector.tensor_copy(
                vaug[b][j][:].rearrange("p (h m) -> p h m", h=HPC)[:, :, 0:64],
                ps[:, 0:128].rearrange("p (h m) -> p h m", h=HPC))
            yield

        def gen_outproj(k):
            """Full 1024-ch output projection for my 128-token strip k.
            og loads use plain row-slice reads: the dep tracker misses
            rearranged DRAM views, letting the read race the collective."""
            og = og_pool.tile([128, 8 * 128], BF16, tag="og")
            for c in range(8):
                nc.sync.dma_start(
                    og[:, 128 * c: 128 * (c + 1)],
                    orecv_d[k][128 * c: 128 * (c + 1), :])
            yield
            for oc in range(2):
                ps = mm_ps.tile([128, 512], F32, tag="mm")
                for c in range(4):
                    nc.tensor.matmul(
                        ps[:],
                        og[:, 128 * c: 128 * (c + 1)],
                        wot[:, 1024 * c + 512 * oc: 1024 * c + 512 * oc + 512],
                        start=(c == 0), stop=False)
                yield
                for c in range(4, 8):
                    nc.tensor.matmul(
                        ps[:],
                        og[:, 128 * c: 128 * (c + 1)],
                        wot[:, 1024 * c + 512 * oc: 1024 * c + 512 * oc + 512],
                        start=False, stop=(c == 7))
                fo = fin_pool.tile([128, 512], F32, tag="fo")
                nc.vector.tensor_add(
                    fo[:], ps[:], bias_sb[:, 512 * oc: 512 * oc + 512])
                nc.gpsimd.dma_start(
                    out_d[128 * k: 128 * (k + 1), 512 * oc: 512 * oc + 512],
                    fo[:])
                yield

        # ---- deadline-driven filler scheduler ----
        class Unit:
            __slots__ = ("deadline", "release", "gen", "started")

            def __init__(self, deadline, gen, release):
                self.deadline = deadline
                self.release = release
                self.gen = gen
                self.started = False

        fillq = []

        def add_unit(deadline, gen, release=0):
            """release = earliest pipeline slot the unit may START at. This
            is a correctness gate, not just pacing: a consumer of a
            collective's output must be EMITTED after the collective is, or
            the dep tracker sees no writer and wires no wait at all."""
            fillq.append(Unit(deadline, gen, release))
            fillq.sort(key=lambda u: u.deadline)

        def advance_fill(n, slot):
            """Advance filler units by n slots. A started (mid-PSUM-chain)
            unit always runs to its next yield before any other unit, so the
            single mm PSUM buffer is never claimed by two chains at once."""
            done = 0
            while done < n and fillq:
                unit = next((u for u in fillq if u.started), None)
                if unit is None:
                    unit = next((u for u in fillq if u.release <= slot), None)
                if unit is None:
                    return
                try:
                    unit.started = True
                    next(unit.gen)
                except StopIteration:
                    fillq.remove(unit)
                    continue
                done += 1

        # ---- softmax normalize: PSUM exits via one DVE copy per head
        # (freeing the AV accumulator fast); reciprocal on DVE; broadcast
        # across partitions via a tiny K=1 PE matmul (ones1 outer product)
        def normalize(b, k, ots):
            c0 = 2048 * b + BW * k
            for e in range(2):
                osb = nrm_pool.tile([65, BW], F32, tag="osb")
                nc.vector.tensor_copy(osb[:], ots[e][:])
                dsb = nrm_pool.tile([1, BW], F32, tag="dsb")
                nc.vector.tensor_copy(dsb[:], osb[64:65, :])
                rsb = nrm_pool.tile([1, BW], F32, tag="rsb")
                nc.vector.reciprocal_approx_fast(rsb[:], dsb[:])
                rsr = nrm_pool.tile([1, BW], BF16, tag="rsr")
                nc.vector.tensor_copy(rsr[:], rsb[:])
                bps = bps_ps.tile([64, BW], F32, tag="bps")
                nc.tensor.matmul(bps[:], ones1[:], rsr[:],
                                 start=True, stop=True)
                bsb = nrm_pool.tile([64, BW], F32, tag="bsb")
                nc.vector.tensor_copy(bsb[:], bps[:])
                nc.vector.tensor_mul(
                    opair[64 * e: 64 * e + 64, c0: c0 + BW],
                    osb[0:64, :], bsb[:])

        def o_send(k):
            """Ship O tiles for strip k to peers + AllToAll. Chunk p (rows
            [128p:+128]) = my channels for owner-core p's strip-k tokens.
            Plain row-slice DMA writes only: the dep tracker misses writes
            through rearranged DRAM views, letting the collective race."""
            for p in range(8):
                bp, gp = divmod(p, 4)
                src0 = 2048 * bp + BW * k + 128 * gp
                nc.sync.dma_start(
                    osend_d[k][128 * p: 128 * (p + 1), :],
                    opair[:, src0: src0 + 128])
            nc.gpsimd.collective_compute(
                "AllToAll", mybir.AluOpType.bypass, replica_groups=GROUP,
                ins=[osend_d[k][:]], outs=[orecv_d[k][:]])

        # ---- static filler schedule ----
        BLOCKS = [(b, k) for b in range(B) for k in range(NBLK)]
        # batch-0 prerequisites, just in time inside block (0,0):
        for j in range(NJ):                      # V tiles for batch 0
            add_unit(j, gen_v(0, j))
        for m in range(1, 4):                    # K cols beyond ic0, batch 0
            add_unit(4 * m - 3, gen_qk_ic(wkt, ktp, 0, m))
        for k in range(1, 4):                    # Q cols for blocks (0,1..3)
            add_unit(16 * k - 5, gen_qk_ic(wqt, qtp, 0, k))
        # batch-1 projections spread across blocks (0,1)-(0,2):
        for m in range(4):
            add_unit(18 + 2 * m, gen_qk_ic(wkt, ktp, 1, m))
        add_unit(26, gen_qk_ic(wqt, qtp, 1, 0))
        for k in range(1, 4):
            add_unit(30 + 4 * k, gen_qk_ic(wqt, qtp, 1, k))
        # batch-1 V tiles land late (deadline: AVs of block (1,0)):
        for j in range(NJ):
            add_unit(50 + j, gen_v(1, j))
        # output projections for strips 0,1. AllToAll #k is EMITTED at slot
        # 16k+82 and takes ~18us (~14 slots); release the og load just after
        # the data lands so the (properly wired) completion wait is ~free:
        add_unit(98, gen_outproj(0), release=98)
        add_unit(114, gen_outproj(1), release=114)

        # ---- the pipeline ----
        pend = []          # (slot_emitted, b, k, j, st_sb)
        ots_map = {}       # (b,k) -> [ots_e0, ots_e1]
        slot = 0

        def emit_scores_exp(b, k, j):
            st_ps = st_ps_pool.tile([128, 1024], F32, tag="st")
            for e in range(2):
                r0 = 64 * e
                nc.tensor.matmul(
                    st_ps[:, 512 * e: 512 * e + 512],
                    ktp[r0:r0 + 64,
                        2048 * b + 128 * j: 2048 * b + 128 * (j + 1)],
                    qtp[r0:r0 + 64, 2048 * b + BW * k: 2048 * b + BW * k + BW],
                    start=True, stop=True)
            st_sb = st_pool.tile([128, 1024], BF16, tag="st")
            nc.scalar.activation(
                st_sb[:], st_ps[:],
                mybir.ActivationFunctionType.Exp, scale=scale)
            return st_sb

        def emit_av(b, k, j, st_sb):
            ots = ots_map.get((b, k))
            if ots is None:
                ots = [ot_ps.tile([65, BW], F32, tag="ot",
                                  name=f"ot{b}_{k}_{e}") for e in range(2)]
                ots_map[(b, k)] = ots
            for e in range(2):
                nc.tensor.matmul(
                    ots[e][:],
                    vaug[b][j][:, 65 * e: 65 * e + 65],
                    st_sb[:, 512 * e: 512 * e + 512],
                    start=(j == 0), stop=(j == NJ - 1))
            if j == NJ - 1:
                normalize(b, k, ots)
                del ots_map[(b, k)]
                if b == 1:
                    o_send(k)

        # prologue PE: q/k ic0 of batch 0 (feeds the first 4 j-iterations)
        for g in (gen_qk_ic(wqt, qtp, 0, 0), gen_qk_ic(wkt, ktp, 0, 0)):
            for _ in g:
                pass

        for bi, (b, k) in enumerate(BLOCKS):
            for j in range(NJ):
                dummy_mm(1 if bi < 4 else 2)
                st_sb = emit_scores_exp(b, k, j)
                pend.append((slot, b, k, j, st_sb))
                while pend and slot - pend[0][0] >= AV_LAG:
                    _, pb, pk, pj, psb = pend.pop(0)
                    emit_av(pb, pk, pj, psb)
                advance_fill(2 if bi == 0 else 1, slot)
                slot += 1

        # epilogue: drain AVs (incl. normalize + o_send of the last strip),
        # then the last two output projections
        for (_, pb, pk, pj, psb) in pend:
            emit_av(pb, pk, pj, psb)
        pend.clear()
        for gen in (gen_outproj(2), gen_outproj(3)):
            for _ in gen:
                pass
        advance_fill(10 ** 6, 10 ** 6)

    nc.compile()
    return nc


def make_in_maps(x, wq, wk, wv, wo, bo):
    """Host-side sharding + layout prep (slices/transposes/dtype only)."""
    bf = ml_dtypes.bfloat16
    x = np.asarray(x, dtype=np.float32)
    bo_b = np.ascontiguousarray(
        np.broadcast_to(np.asarray(bo, np.float32)[None, :], (128, D)))
    wq, wk, wv, wo = (np.asarray(w, np.float32) for w in (wq, wk, wv, wo))
    # x^T with both batches side by side: [D, 4096], col u = 2048*b + t
    xt = np.ascontiguousarray(x.transpose(2, 0, 1).reshape(D, U).astype(bf))
    wot_full = np.ascontiguousarray(wo.T.astype(bf))
    in_maps = []
    for c in range(N_CORES):
        r0 = CPC * c
        in_maps.append({
            "xt": xt,
            "wqt": np.ascontiguousarray(wq[r0:r0 + CPC, :].T.astype(bf)),
            "wkt": np.ascontiguousarray(wk[r0:r0 + CPC, :].T.astype(bf)),
            "wvt": np.ascontiguousarray(wv[r0:r0 + CPC, :].T.astype(bf)),
            "wot": wot_full,
            "bob": bo_b,
        })
    return in_maps


_PROG_CACHE = {}


def _get_prog():
    if "prog" not in _PROG_CACHE:
        _PROG_CACHE["prog"] = build_program()
    return _PROG_CACHE["prog"]


def run(x, wq, wk, wv, wo, bo, trace=False, trace_cores=None):
    """Run on hardware; returns (output [B,N,D], exec_time_ns or None)."""
    from concourse.bass_utils import run_bass_kernel_spmd

    nc = _get_prog()
    in_maps = make_in_maps(x, wq, wk, wv, wo, bo)
    kw = {}
    if trace:
        kw = dict(trace=True, trace_cores=trace_cores or [0])
    res = run_bass_kernel_spmd(nc, in_maps, list(range(N_CORES)), **kw)
    out = np.empty((B, N, D), dtype=np.float32)
    for c in range(N_CORES):
        b, g = divmod(c, 4)
        o = res.results[c]["out"]
        for k in range(NBLK):
            t0 = BW * k + 128 * g
            out[b, t0:t0 + 128, :] = o[128 * k: 128 * (k + 1)]
    return out, res.exec_time_ns


def kernel(x, wq, wk, wv, wo, bo):
    out, _ = run(x, wq, wk, wv, wo, bo)
    return out
